# revision 20
# baseline (speedup 1.0000x reference)
"""ClinicalSafetyLoss Trainium2 kernel (v3 — DVE+ACT only, min instruction count).

Computes  loss = CE + 0.3*safety_penalty + 0.5*critical_penalty  over
outputs [B,3] f32 / targets [B] i64, B = 4_194_304, data-parallel over 8
NeuronCores (batch-sharded), per-core partial sums combined on host.

Math (per row, x0,x1,x2 logits, t target):
    d01 = x0 - x1;  d12 = x2 - x1
    LL  = ln(1 + e^d01 + e^d12) = lse - x1
    ce_i = LL - [t==0]*d01 - [t==2]*d12          [x1 cancels]
    q = [pred==0] + [pred!=2]  in {0,1,2}   (exact first-max argmax; pred = 2-q)
    pen = P[t,pred] = w(t)*[q>1] + (2 - t - q) + 11*miss
      with w(t) = 6*[t>=1]+5*[t>=2] = t*(6.5-0.5t)  and  g1+g2 == t on {0,1,2}
    miss = [t>=2]*[q>=1];  G2 = sum relu(t-1)

Engine split per tile (GpSimd untouched — it shares an exclusive SBUF port
pair with the DVE and would block it):
  DVE:  dd (paged subtract), S=e0+e1 (bf16 2x), q (custom), Va (custom,
        accum), miss=[q>=1]*g2tile (bf16 STT, accum), X=[t==0]d01+[t>=2]d12
        (paged custom, accum)
  ACT:  exp(dd) bf16, ln(S+1) (accum), g2=relu(t-1) (accum)
Host: pen_sum = sum(Va) + 2B + 11*miss;  one output DMA at the end.
"""

import numpy as np

B_TOTAL = 4_194_304
N_CORES = 8
BC = B_TOTAL // N_CORES          # rows per core = 524_288
P = 128                          # SBUF partitions
# Progressive ramp: per-tile DMA fill (~7.4ns/row-col) stays ahead of DVE
# compute (~8.5ns/row-col) as long as K grows by <=~1.15x per step; big middle
# tiles amortize per-instruction fixed cost; small last tile for the tail.
K_SCHED = [256, 512, 1024, 1280, 768, 256]
T = len(K_SCHED)

N_ACC = 5                        # Va, M, X, LL, G2 columns per tile

_STATE: dict = {}


def _register_dve_ops():
    """Register the fused vector-engine ops (runtime append to the custom-DVE
    registry; sha computed locally so compile's drift check passes)."""
    import concourse.dve_ops as dvo
    from concourse.dve_spec import (
        Spec, Src0, Src1, SubIdx, Zero, One, C0, C1, C2, select, lower,
    )
    from concourse.dve_spec import _has_src1
    from concourse.dve_uop import DveOpSpec
    from operator import add

    def mk(name, spec, subdim=False):
        for o in dvo.OPS:
            if o.name == name:
                return o
        shas = {}
        for ver in ("v3", "v4"):
            uops = lower(spec, ver=ver)
            shas[ver] = DveOpSpec(
                name=name, opcode=0, uops=uops, rd1_en=_has_src1(spec)
            ).sha(ver)
        op = dvo.DveOp(name, spec, subdim=subdim, uops_sha=shas)
        dvo.OPS.append(op)
        dvo.CUSTOM_DVE_SPECS[name] = spec
        dvo._SUB_OPCODE_FOR_NAME[name] = dvo._CUSTOM_DVE_ROW_BASE + len(dvo.OPS) - 1
        return op

    # q = [pred==0] + [pred!=2] = p0 + np2  (in0=d01, in1=d12), exact
    # first-max argmax semantics.
    c1 = Src0 >= Zero
    c2 = Src0 >= Src1
    op_q = mk("CSL_Q3", Spec(
        body=c1 * c2 + select(c1, c2, Src1 <= Zero),
        reference=lambda in0, in1, s0, s1, imm2:
            (((in0 >= 0) & (in0 >= in1)).astype(np.float32)
             + np.where(in0 >= 0, in0 >= in1, in1 <= 0).astype(np.float32)),
    ))

    # Va = w(t)*[q>1] - (t+q)  (in0=t, in1=q, s0=6.5, imm2=0.5); accum add.
    # pen = Va + 2 + 11*miss reconstructed on host.
    w = Src0 * (C0 - Src0 * C2)
    body_va = (w * (One < Src1)) - (Src0 + Src1)

    def _va_ref(in0, in1, s0, s1, imm2):
        t = np.asarray(in0, dtype=np.float32)
        q = np.asarray(in1, dtype=np.float32)
        b = ((t * (s0 - t * imm2)) * (q > 1) - (t + q)).astype(np.float32)
        return b, b.reshape(b.shape[0], -1).sum(axis=-1, keepdims=True)

    op_va = mk("CSL_VA", Spec(body=body_va, accum=add, reference=_va_ref))

    # X paged over dd ([P,2,K]; page 0 = d01, page 1 = d12):
    #   page 0: [t == 0]*d01,  page 1: [t >= 2]*d12; accum add
    # in0 = t broadcast [P,2,K], in1 = dd, s1 = 2.0
    def _xt_ref(in0, in1, s0, s1, imm2):
        j = np.zeros_like(np.asarray(in0, dtype=np.float32))
        j[:, 1:, :] = 1.0
        b = (np.where(j >= 1, in0 >= s1, in0 < 1).astype(np.float32) * in1)
        return b.astype(np.float32), b.reshape(b.shape[0], -1).sum(-1, keepdims=True)

    op_xt = mk("CSL_XT", Spec(
        body=select(SubIdx >= One, Src0 >= C1, Src0 < One) * Src1,
        accum=add,
        reference=_xt_ref,
    ), subdim=True)

    # dd = x02 - x11 as a custom op: the custom-DVE path streams the paged
    # broadcast pattern at 1 elem/cycle where the stock TensorTensor pays ~1.3x.
    op_dd = mk("CSL_DD", Spec(
        body=Src0 - Src1,
        reference=lambda in0, in1, s0, s1, imm2:
            (np.asarray(in0, np.float32) - np.asarray(in1, np.float32)),
    ))
    return op_q, op_va, op_xt, op_dd


def _build():
    """Trace + compile the per-core Bass program. Returns the finalized nc."""
    import concourse.bacc as bacc
    import concourse.mybir as mybir
    import concourse.tile as tile

    op_q, op_va, op_xt, op_dd = _register_dve_ops()

    f32 = mybir.dt.float32
    bf16 = mybir.dt.bfloat16
    i32 = mybir.dt.int32
    Alu = mybir.AluOpType
    Act = mybir.ActivationFunctionType

    nc = bacc.Bacc("TRN2", target_bir_lowering=False, debug=False)

    # Pin Exp/Ln/Relu to the one ACT table set that holds them all so the
    # per-tile func mix never thrashes ACT_TABLE_LOADs.
    from concourse.hw_specs import get_activation_tables
    tabs = get_activation_tables(nc.m.arch)
    for name, funcs in tabs.items():
        if name != "natural_log_exp_and_others":
            for fn in (Act.Exp, Act.Ln, Act.Relu, Act.Identity, Act.Copy):
                funcs.discard(fn)

    # const AP for the Relu(t - 1) bias
    _nb = nc.alloc_sbuf_tensor("const-f32-neg1", [P, 1], f32)
    nc.gpsimd.memset(_nb.ap(), -1.0)
    nc.const_aps.aps[(f32, -1.0)] = _nb.ap()

    x_dram = nc.dram_tensor("x", [BC, 3], f32, kind="ExternalInput")
    t_dram = nc.dram_tensor("t", [BC, 2], i32, kind="ExternalInput")  # int64 lo/hi
    acc_dram = nc.dram_tensor("acc", [P, T * N_ACC], f32, kind="ExternalOutput")

    assert sum(K_SCHED) == BC // P

    with tile.TileContext(nc) as tc:
        with (
            tc.tile_pool(name="xin", bufs=3) as xpool,
            tc.tile_pool(name="tin", bufs=3) as tpool,
            tc.tile_pool(name="work", bufs=2) as wpool,
            tc.tile_pool(name="accp", bufs=1) as apool,
        ):
            acc = apool.tile([P, T * N_ACC], f32, tag="acc")

            row_off = 0
            for it, K in enumerate(K_SCHED):
                xt = xpool.tile([P, K, 3], f32, tag="x")
                tt = tpool.tile([P, K, 2], i32, tag="t")
                x_src = x_dram[row_off: row_off + P * K].rearrange(
                    "(p k) c -> p k c", p=P, k=K)
                t_src = t_dram[row_off: row_off + P * K].rearrange(
                    "(p k) w -> p k w", p=P, k=K)
                nc.sync.dma_start(xt[:], x_src)
                nc.sync.dma_start(tt[:], t_src)
                row_off += P * K

                tl = tt[:, :, 0]          # low int32 word of each int64 target
                a = lambda j: acc[:, it * N_ACC + j: it * N_ACC + j + 1]

                # dd[:,0,:] = x0-x1, dd[:,1,:] = x2-x1 in one paged pass.
                x02 = xt[:, :, 0:3:2].rearrange("p k j -> p j k")
                x11 = xt[:, :, 1:2].rearrange("p k j -> p j k").to_broadcast([P, 2, K])
                dd = wpool.tile([P, 2, K], f32, tag="dd")
                nc.vector._custom_dve(op_dd, out=dd[:], in0=x02, in1=x11)
                d01 = dd[:, 0, :]
                d12 = dd[:, 1, :]

                # --- CE path: LL = ln(1 + e^d01 + e^d12); exp in bf16 (the
                # ~0.1% rounding is zero-mean over 4M rows, <1e-6 on the loss).
                ee = wpool.tile([P, 2, K], bf16, tag="ee")
                nc.scalar.activation(ee[:], dd[:], Act.Exp)
                S = wpool.tile([P, K], bf16, tag="S")
                nc.vector.tensor_tensor(S[:], ee[:, 0, :], ee[:, 1, :], Alu.add)
                LL = wpool.tile([P, K], bf16, tag="LL")
                nc.scalar.activation(LL[:], S[:], Act.Ln, bias=1.0, accum_out=a(3))

                # --- critical-class tile + count: g2 = relu(t - 1) ---
                G2t = wpool.tile([P, K], bf16, tag="G2")
                nc.scalar.activation(G2t[:], tl, Act.Relu, bias=-1.0,
                                     accum_out=a(4))

                # --- pred class q, penalty Va, miss (DVE) ---
                qv = wpool.tile([P, K], bf16, tag="q")
                nc.vector._custom_dve(op_q, out=qv[:], in0=d01, in1=d12)
                Vv = wpool.tile([P, K], bf16, tag="V")
                nc.vector._custom_dve(op_va, out=Vv[:], in0=tl, in1=qv[:],
                                      s0=6.5, imm2=0.5, accum_out=a(0))
                # miss = [q>=1]*g2 — all-bf16 STT runs in 2x mode
                Mv = wpool.tile([P, K], bf16, tag="M")
                nc.vector.scalar_tensor_tensor(Mv[:], qv[:], 1.0, G2t[:],
                                               Alu.is_ge, Alu.mult, accum_out=a(1))

                # --- x_t pieces: one paged pass over dd ---
                trep = tt[:, :, 0:1].rearrange("p k j -> p j k").to_broadcast([P, 2, K])
                xv = wpool.tile([P, 2, K], f32, tag="xv")
                nc.vector._custom_dve(op_xt, out=xv[:], in0=trep, in1=dd[:],
                                      s1=2.0, accum_out=a(2))

            # Single tiny result DMA at the very end.
            nc.sync.dma_start(acc_dram[:, :], acc[:, :])

    nc.compile()
    return nc


def _ensure_built():
    if "nc" not in _STATE:
        _STATE["nc"] = _build()
    return _STATE["nc"]


def _combine(results):
    """Host-side float64 combine of the per-core accumulators into the loss."""
    SVa = miss = SX = SLL = SG2 = 0.0
    for r in results:
        acc = r["acc"].astype(np.float64).reshape(P, T, N_ACC)
        SVa += acc[:, :, 0].sum()
        miss += acc[:, :, 1].sum()
        SX += acc[:, :, 2].sum()
        SLL += acc[:, :, 3].sum()
        SG2 += acc[:, :, 4].sum()

    B = float(B_TOTAL)
    ce_sum = SLL - SX
    pen_sum = SVa + 2.0 * B + 11.0 * miss
    critical = 10.0 * miss / max(SG2, 1.0) if SG2 > 0 else 0.0
    loss = ce_sum / B + 0.3 * pen_sum / B + critical
    return np.asarray(loss, dtype=np.float32)


def kernel(outputs: np.ndarray, targets: np.ndarray) -> np.ndarray:
    import os
    from concourse.bass_utils import run_bass_kernel_spmd

    nc = _ensure_built()

    x = np.ascontiguousarray(np.asarray(outputs, dtype=np.float32)).reshape(
        N_CORES, BC, 3)
    t64 = np.ascontiguousarray(np.asarray(targets).astype(np.int64, copy=False))
    t32 = t64.view(np.int32).reshape(N_CORES, BC, 2)

    in_maps = [{"x": x[c], "t": t32[c]} for c in range(N_CORES)]
    trace = bool(int(os.environ.get("CSL_TRACE", "0")))
    tmpdir = os.environ.get("CSL_TRACE_DIR") or None
    res = run_bass_kernel_spmd(nc, in_maps, list(range(N_CORES)), trace=trace,
                               tmpdir=tmpdir)
    kernel._last_exec_time_ns = getattr(res, "exec_time_ns", None)
    return _combine(res.results)


kernel._last_exec_time_ns = None


# revision 21
# speedup vs baseline: 1.1649x; 1.1649x over previous
"""ClinicalSafetyLoss Trainium2 kernel (v3 — DVE+ACT only, min instruction count).

Computes  loss = CE + 0.3*safety_penalty + 0.5*critical_penalty  over
outputs [B,3] f32 / targets [B] i64, B = 4_194_304, data-parallel over 8
NeuronCores (batch-sharded), per-core partial sums combined on host.

Math (per row, x0,x1,x2 logits, t target):
    d01 = x0 - x1;  d12 = x2 - x1
    LL  = ln(1 + e^d01 + e^d12) = lse - x1
    ce_i = LL - [t==0]*d01 - [t==2]*d12          [x1 cancels]
    q = [pred==0] + [pred!=2]  in {0,1,2}   (exact first-max argmax; pred = 2-q)
    pen = P[t,pred] = w(t)*[q>1] + (2 - t - q) + 11*miss
      with w(t) = 6*[t>=1]+5*[t>=2] = t*(6.5-0.5t)  and  g1+g2 == t on {0,1,2}
    miss = [t>=2]*[q>=1];  G2 = sum relu(t-1)

Engine split per tile (GpSimd untouched — it shares an exclusive SBUF port
pair with the DVE and would block it):
  DVE:  dd (paged subtract), S=e0+e1 (bf16 2x), q (custom), Va (custom,
        accum), miss=[q>=1]*g2tile (bf16 STT, accum), X=[t==0]d01+[t>=2]d12
        (paged custom, accum)
  ACT:  exp(dd) bf16, ln(S+1) (accum), g2=relu(t-1) (accum)
Host: pen_sum = sum(Va) + 2B + 11*miss;  one output DMA at the end.
"""

import numpy as np

B_TOTAL = 4_194_304
N_CORES = 8
BC = B_TOTAL // N_CORES          # rows per core = 524_288
P = 128                          # SBUF partitions
# Progressive ramp: per-tile DMA fill (~7.4ns/row-col) stays ahead of DVE
# compute (~8.5ns/row-col) as long as K grows by <=~1.15x per step; big middle
# tiles amortize per-instruction fixed cost; small last tile for the tail.
K_SCHED = [256, 640, 1152, 1408, 640]
T = len(K_SCHED)

N_ACC = 5                        # Va, M, X, LL, G2 columns per tile

_STATE: dict = {}


def _register_dve_ops():
    """Register the fused vector-engine ops (runtime append to the custom-DVE
    registry; sha computed locally so compile's drift check passes)."""
    import concourse.dve_ops as dvo
    from concourse.dve_spec import (
        Spec, Src0, Src1, SubIdx, Zero, One, C0, C1, C2, select, lower,
    )
    from concourse.dve_spec import _has_src1
    from concourse.dve_uop import DveOpSpec
    from operator import add

    def mk(name, spec, subdim=False):
        for o in dvo.OPS:
            if o.name == name:
                return o
        shas = {}
        for ver in ("v3", "v4"):
            uops = lower(spec, ver=ver)
            shas[ver] = DveOpSpec(
                name=name, opcode=0, uops=uops, rd1_en=_has_src1(spec)
            ).sha(ver)
        op = dvo.DveOp(name, spec, subdim=subdim, uops_sha=shas)
        dvo.OPS.append(op)
        dvo.CUSTOM_DVE_SPECS[name] = spec
        dvo._SUB_OPCODE_FOR_NAME[name] = dvo._CUSTOM_DVE_ROW_BASE + len(dvo.OPS) - 1
        return op

    # q = [pred==0] + [pred!=2] = p0 + np2  (in0=d01, in1=d12), exact
    # first-max argmax semantics.
    c1 = Src0 >= Zero
    c2 = Src0 >= Src1
    op_q = mk("CSL_Q3", Spec(
        body=c1 * c2 + select(c1, c2, Src1 <= Zero),
        reference=lambda in0, in1, s0, s1, imm2:
            (((in0 >= 0) & (in0 >= in1)).astype(np.float32)
             + np.where(in0 >= 0, in0 >= in1, in1 <= 0).astype(np.float32)),
    ))

    # Va = w(t)*[q>1] - (t+q)  (in0=t, in1=q, s0=6.5, imm2=0.5); accum add.
    # pen = Va + 2 + 11*miss reconstructed on host.
    w = Src0 * (C0 - Src0 * C2)
    body_va = (w * (One < Src1)) - (Src0 + Src1)

    def _va_ref(in0, in1, s0, s1, imm2):
        t = np.asarray(in0, dtype=np.float32)
        q = np.asarray(in1, dtype=np.float32)
        b = ((t * (s0 - t * imm2)) * (q > 1) - (t + q)).astype(np.float32)
        return b, b.reshape(b.shape[0], -1).sum(axis=-1, keepdims=True)

    op_va = mk("CSL_VA", Spec(body=body_va, accum=add, reference=_va_ref))

    # X paged over dd ([P,2,K]; page 0 = d01, page 1 = d12):
    #   page 0: [t == 0]*d01,  page 1: [t >= 2]*d12; accum add
    # in0 = t broadcast [P,2,K], in1 = dd, s1 = 2.0
    def _xt_ref(in0, in1, s0, s1, imm2):
        j = np.zeros_like(np.asarray(in0, dtype=np.float32))
        j[:, 1:, :] = 1.0
        b = (np.where(j >= 1, in0 >= s1, in0 < 1).astype(np.float32) * in1)
        return b.astype(np.float32), b.reshape(b.shape[0], -1).sum(-1, keepdims=True)

    op_xt = mk("CSL_XT", Spec(
        body=select(SubIdx >= One, Src0 >= C1, Src0 < One) * Src1,
        accum=add,
        reference=_xt_ref,
    ), subdim=True)

    # dd = x02 - x11 as a custom op: the custom-DVE path streams the paged
    # broadcast pattern at 1 elem/cycle where the stock TensorTensor pays ~1.3x.
    op_dd = mk("CSL_DD", Spec(
        body=Src0 - Src1,
        reference=lambda in0, in1, s0, s1, imm2:
            (np.asarray(in0, np.float32) - np.asarray(in1, np.float32)),
    ))
    return op_q, op_va, op_xt, op_dd


def _build():
    """Trace + compile the per-core Bass program. Returns the finalized nc."""
    import concourse.bacc as bacc
    import concourse.mybir as mybir
    import concourse.tile as tile

    op_q, op_va, op_xt, op_dd = _register_dve_ops()

    f32 = mybir.dt.float32
    bf16 = mybir.dt.bfloat16
    i32 = mybir.dt.int32
    Alu = mybir.AluOpType
    Act = mybir.ActivationFunctionType

    nc = bacc.Bacc("TRN2", target_bir_lowering=False, debug=False)

    # Pin Exp/Ln/Relu to the one ACT table set that holds them all so the
    # per-tile func mix never thrashes ACT_TABLE_LOADs.
    from concourse.hw_specs import get_activation_tables
    tabs = get_activation_tables(nc.m.arch)
    for name, funcs in tabs.items():
        if name != "natural_log_exp_and_others":
            for fn in (Act.Exp, Act.Ln, Act.Relu, Act.Identity, Act.Copy):
                funcs.discard(fn)

    # const AP for the Relu(t - 1) bias
    _nb = nc.alloc_sbuf_tensor("const-f32-neg1", [P, 1], f32)
    nc.gpsimd.memset(_nb.ap(), -1.0)
    nc.const_aps.aps[(f32, -1.0)] = _nb.ap()

    x_dram = nc.dram_tensor("x", [BC, 3], f32, kind="ExternalInput")
    t_dram = nc.dram_tensor("t", [BC, 2], i32, kind="ExternalInput")  # int64 lo/hi
    acc_dram = nc.dram_tensor("acc", [P, T * N_ACC], f32, kind="ExternalOutput")

    assert sum(K_SCHED) == BC // P

    with tile.TileContext(nc) as tc:
        with (
            tc.tile_pool(name="xin", bufs=3) as xpool,
            tc.tile_pool(name="tin", bufs=3) as tpool,
            tc.tile_pool(name="work", bufs=2) as wpool,
            tc.tile_pool(name="accp", bufs=1) as apool,
        ):
            acc = apool.tile([P, T * N_ACC], f32, tag="acc")

            row_off = 0
            for it, K in enumerate(K_SCHED):
                xt = xpool.tile([P, K, 3], f32, tag="x")
                tt = tpool.tile([P, K, 2], i32, tag="t")
                x_src = x_dram[row_off: row_off + P * K].rearrange(
                    "(p k) c -> p k c", p=P, k=K)
                t_src = t_dram[row_off: row_off + P * K].rearrange(
                    "(p k) w -> p k w", p=P, k=K)
                nc.sync.dma_start(xt[:], x_src)
                nc.sync.dma_start(tt[:], t_src)
                row_off += P * K

                tl = tt[:, :, 0]          # low int32 word of each int64 target
                a = lambda j: acc[:, it * N_ACC + j: it * N_ACC + j + 1]

                # dd[:,0,:] = x0-x1, dd[:,1,:] = x2-x1 in one paged pass.
                x02 = xt[:, :, 0:3:2].rearrange("p k j -> p j k")
                x11 = xt[:, :, 1:2].rearrange("p k j -> p j k").to_broadcast([P, 2, K])
                dd = wpool.tile([P, 2, K], f32, tag="dd")
                nc.vector._custom_dve(op_dd, out=dd[:], in0=x02, in1=x11)
                d01 = dd[:, 0, :]
                d12 = dd[:, 1, :]

                # --- CE path: LL = ln(1 + e^d01 + e^d12); exp in bf16 (the
                # ~0.1% rounding is zero-mean over 4M rows, <1e-6 on the loss).
                ee = wpool.tile([P, 2, K], bf16, tag="ee")
                nc.scalar.activation(ee[:], dd[:], Act.Exp)
                S = wpool.tile([P, K], bf16, tag="S")
                nc.vector.tensor_tensor(S[:], ee[:, 0, :], ee[:, 1, :], Alu.add)
                LL = wpool.tile([P, K], bf16, tag="LL")
                nc.scalar.activation(LL[:], S[:], Act.Ln, bias=1.0, accum_out=a(3))

                # --- critical-class tile + count: g2 = relu(t - 1) ---
                G2t = wpool.tile([P, K], bf16, tag="G2")
                nc.scalar.activation(G2t[:], tl, Act.Relu, bias=-1.0,
                                     accum_out=a(4))

                # --- pred class q, penalty Va, miss (DVE) ---
                qv = wpool.tile([P, K], bf16, tag="q")
                nc.vector._custom_dve(op_q, out=qv[:], in0=d01, in1=d12)
                Vv = wpool.tile([P, K], bf16, tag="V")
                nc.vector._custom_dve(op_va, out=Vv[:], in0=tl, in1=qv[:],
                                      s0=6.5, imm2=0.5, accum_out=a(0))
                # miss = [q>=1]*g2 — all-bf16 STT runs in 2x mode
                Mv = wpool.tile([P, K], bf16, tag="M")
                nc.vector.scalar_tensor_tensor(Mv[:], qv[:], 1.0, G2t[:],
                                               Alu.is_ge, Alu.mult, accum_out=a(1))

                # --- x_t pieces: one paged pass over dd ---
                trep = tt[:, :, 0:1].rearrange("p k j -> p j k").to_broadcast([P, 2, K])
                xv = wpool.tile([P, 2, K], f32, tag="xv")
                nc.vector._custom_dve(op_xt, out=xv[:], in0=trep, in1=dd[:],
                                      s1=2.0, accum_out=a(2))

            # Single tiny result DMA at the very end.
            nc.sync.dma_start(acc_dram[:, :], acc[:, :])

    nc.compile()
    return nc


def _ensure_built():
    if "nc" not in _STATE:
        _STATE["nc"] = _build()
    return _STATE["nc"]


def _combine(results):
    """Host-side float64 combine of the per-core accumulators into the loss."""
    SVa = miss = SX = SLL = SG2 = 0.0
    for r in results:
        acc = r["acc"].astype(np.float64).reshape(P, T, N_ACC)
        SVa += acc[:, :, 0].sum()
        miss += acc[:, :, 1].sum()
        SX += acc[:, :, 2].sum()
        SLL += acc[:, :, 3].sum()
        SG2 += acc[:, :, 4].sum()

    B = float(B_TOTAL)
    ce_sum = SLL - SX
    pen_sum = SVa + 2.0 * B + 11.0 * miss
    critical = 10.0 * miss / max(SG2, 1.0) if SG2 > 0 else 0.0
    loss = ce_sum / B + 0.3 * pen_sum / B + critical
    return np.asarray(loss, dtype=np.float32)


def kernel(outputs: np.ndarray, targets: np.ndarray) -> np.ndarray:
    import os
    from concourse.bass_utils import run_bass_kernel_spmd

    nc = _ensure_built()

    x = np.ascontiguousarray(np.asarray(outputs, dtype=np.float32)).reshape(
        N_CORES, BC, 3)
    t64 = np.ascontiguousarray(np.asarray(targets).astype(np.int64, copy=False))
    t32 = t64.view(np.int32).reshape(N_CORES, BC, 2)

    in_maps = [{"x": x[c], "t": t32[c]} for c in range(N_CORES)]
    trace = bool(int(os.environ.get("CSL_TRACE", "0")))
    tmpdir = os.environ.get("CSL_TRACE_DIR") or None
    res = run_bass_kernel_spmd(nc, in_maps, list(range(N_CORES)), trace=trace,
                               tmpdir=tmpdir)
    kernel._last_exec_time_ns = getattr(res, "exec_time_ns", None)
    return _combine(res.results)


kernel._last_exec_time_ns = None
